# revision 12
# baseline (speedup 1.0000x reference)
"""Bidirectional Mamba on 8 Trainium2 NeuronCores.

Sharding: 8 cores = (2 directions) x (4 batch elements); each core runs one
full Mamba block on its (L=1024, DM=512) sequence. The backward direction is
handled by flipping the sequence on the host before/after, so all cores run
the identical SPMD program with different data.

Per-core layout: channels d on partitions, time t on the free dim. The d=512
channels form 4 chunks of 128; chunk pairs are concatenated along the free
dim into (128, 2048) tiles. Engine balance per (h, n) scan iteration:
  - Act: dA = exp(dt*A_n) (fp32, one zeroed decay column at the pair
    boundary kept persistent so the scan restarts for the 2nd chunk)
  - DVE: dBx = u*B_n and hC = h*C_n in bf16 (2x_1p mode, all-SBUF bf16)
  - Pool (GPSIMD): the tensor_tensor_scan itself (fp32 internal state)
  - PE:  y += hC via bf16 identity matmuls into PSUM
B_n/C_n rows are broadcast across partitions by DMA from a bf16 DRAM copy
of dbc; each broadcast is shared by both chunk pairs. The depthwise conv
runs on the PE as K diagonal-matrix matmuls accumulating in PSUM over a
left-zero-padded copy of xc. All dense GEMMs use float32r (1 cycle/row).
"""
import contextlib

import numpy as np

import concourse.bacc as bacc
import concourse.tile as tile
import concourse.mybir as mybir
from concourse.bass_utils import run_bass_kernel_spmd

F32 = mybir.dt.float32
F32R = mybir.dt.float32r
BF16 = mybir.dt.bfloat16
AF = mybir.ActivationFunctionType
OP = mybir.AluOpType

DM = 512
DI = 512
L = 1024
N = 16
K = 4
R = 32
P = 128
NCH = DI // P          # 4 d-chunks
W = 2 * L              # wide tile free size (chunk pair)
TB = 512               # t-block for matmul moving operand (1 PSUM bank)
NTB = L // TB          # 2
N_CORES = 8

# GPSIMD (Pool) supports only TensorTensor; scans must run on DVE. The
# bf16 multiplies are split: hC goes to Pool except where HC_ON_DVE, dBx
# stays on DVE (2x_1p bf16 mode).
HC_ON_DVE = {(h, n) for h in range(2) for n in (5, 11)}


def _mmr(nc, out, lhsT, rhs, start, stop):
    nc.tensor.matmul(out, lhsT=lhsT.bitcast(F32R), rhs=rhs.bitcast(F32R),
                     start=start, stop=stop, skip_group_check=True)


def _mmb(nc, out, lhsT, rhs, start, stop):
    nc.tensor.matmul(out, lhsT=lhsT, rhs=rhs, start=start, stop=stop,
                     skip_group_check=True)


def emit_mamba(tc, io):
    nc = tc.nc
    f32 = F32

    with contextlib.ExitStack() as ctx:
        # ---- persistent SBUF tiles ----
        per = ctx.enter_context(tc.tile_pool(name="per", bufs=1))

        def ptile(tag, shape, dtype=f32):
            return per.tile(shape, dtype, tag=tag, name=tag)

        # packed per-chunk params: [Wc(K) | bconv | Wx(R+2N) | bdt | A(N) | D]
        SP_W = K + 1 + (R + 2 * N) + 1 + N + 1
        o_wc, o_bc, o_wx, o_bdt, o_a, o_d = (
            0, K, K + 1, K + 1 + R + 2 * N, K + 2 + R + 2 * N,
            K + 2 + R + 2 * N + N,
        )
        small_sb = ptile("small", [P, NCH, SP_W])
        Wdt_sb = ptile("Wdt", [R, DI], BF16)
        Wout_sb = ptile("Wout", [P, NCH, DM], BF16)
        Wdiag_sb = ptile("Wdiag", [P, NCH * K * P])
        ident_sb = ptile("ident", [P, P], BF16)

        def Wc(dc):
            return small_sb[:, dc, o_wc:o_bc]

        def bconv(dc):
            return small_sb[:, dc, o_bc:o_bc + 1]

        def Wx(dc):
            return small_sb[:, dc, o_wx:o_bdt]

        def bdt(dc):
            return small_sb[:, dc, o_bdt:o_bdt + 1]

        def A_sc(dc):
            return small_sb[:, dc, o_a:o_a + N]

        def Dv(dc):
            return small_sb[:, dc, o_d:o_d + 1]

        # wide (chunk-pair) activations
        zs_sb = [ptile(f"zs{i}", [P, W]) for i in range(2)]
        xs_sb = [ptile(f"xs{i}", [P, W]) for i in range(2)]
        dt_sb = [ptile(f"dt{i}", [P, W]) for i in range(2)]
        u_sb = [ptile(f"u{i}", [P, W], BF16) for i in range(2)]
        yz_sb = [ptile(f"yz{i}", [P, W], BF16) for i in range(2)]
        dtin_sb = ptile("dtin", [R, L], BF16)
        dbcb_sb = ptile("dbcb", [2 * N, L], BF16)
        # persistent dA tiles: [h][parity]; col L stays 0 (scan restart)
        dA_sb = [[ptile(f"dA{h}{pr}", [P, W]) for pr in range(2)]
                 for h in range(2)]

        def wide(arr, dc, lo=0, hi=L):
            return arr[dc // 2][:, (dc % 2) * L + lo : (dc % 2) * L + hi]

        nc.sync.dma_start(small_sb[:], io["small"][:, :, :])
        nc.sync.dma_start(Wdt_sb[:], io["Wdt"][:, :])
        nc.sync.dma_start(Wout_sb[:], io["W_out"][:, :, :])
        nc.sync.dma_start(Wdiag_sb[:], io["Wdiag"][:, :])
        nc.sync.dma_start(ident_sb[:], io["ident"][:, :])
        for h in range(2):
            for pr in range(2):
                nc.vector.memset(dA_sb[h][pr][:, L : L + 1], 0.0)

        # ---- GEMM A: xz_T = W_in^T @ x_T ; silu on z half ----
        # xc goes into a left-zero-padded tile so the causal conv can run as
        # full-range shifted matmuls.
        with tc.tile_pool(name="gin", bufs=1) as gin, tc.tile_pool(
            name="psA", bufs=2, space="PSUM"
        ) as psA:
            W_in_sb = gin.tile([P, NCH, 2 * DI], F32R, tag="Wi", name="Wi")
            xT_sb = gin.tile([P, NCH, L], F32R, tag="xT", name="xT")
            CPAD = K - 1
            xcp_sb = [
                gin.tile([P, CPAD + L], f32, tag=f"xcp{i}", name=f"xcp{i}")
                for i in range(NCH)
            ]
            nc.sync.dma_start(W_in_sb[:], io["W_in"][:, :, :])
            nc.sync.dma_start(xT_sb[:], io["xT"][:, :, :])
            for i in range(NCH):
                nc.vector.memset(xcp_sb[i][:, 0:CPAD], 0.0)

            for cb in range(2 * DI // P):  # 8 output blocks of 128 channels
                for tb in range(NTB):
                    ps = psA.tile([P, TB], f32, tag="psA", name="psA")
                    for mk in range(NCH):
                        _mmb(
                            nc, ps[:],
                            W_in_sb[:, mk, cb * P : (cb + 1) * P],
                            xT_sb[:, mk, tb * TB : (tb + 1) * TB],
                            start=(mk == 0), stop=(mk == NCH - 1),
                        )
                    lo, hi = tb * TB, (tb + 1) * TB
                    if cb < NCH:
                        nc.scalar.activation(
                            xcp_sb[cb][:, CPAD + lo : CPAD + hi], ps[:],
                            AF.Copy,
                        )
                    else:
                        nc.scalar.activation(
                            wide(zs_sb, cb - NCH, lo, hi), ps[:], AF.Silu
                        )

            # ---- causal depthwise conv (K=4) + silu -> xs ----
            # chunks 0-1 on the PE (fp32 diagonal matmuls, PSUM accumulate,
            # one bank per out block); chunks 2-3 on DVE (fused stt chain).
            with tc.tile_pool(name="pscv", bufs=2, space="PSUM") as pscv, \
                 tc.tile_pool(name="cvv", bufs=2) as cvv:
                for dc in range(NCH):
                    if dc < 2:
                        cps = pscv.tile([P, L], f32, tag="cps", name="cps")
                        for tb in range(NTB):
                            osl = slice(tb * TB, (tb + 1) * TB)
                            for s in range(K):
                                k = K - 1 - s
                                dg = Wdiag_sb[
                                    :, (dc * K + k) * P : (dc * K + k + 1) * P
                                ]
                                _mmb(nc, cps[:, osl], dg,
                                     xcp_sb[dc][:, CPAD - s + tb * TB
                                                 : CPAD - s + tb * TB + TB],
                                     start=(s == 0), stop=(s == K - 1))
                        nc.scalar.activation(
                            wide(xs_sb, dc), cps[:, 0:L], AF.Silu,
                            bias=bconv(dc)[:, 0:1],
                        )
                    else:
                        xcv = cvv.tile([P, L], f32, tag="xcv", name="xcv")
                        nc.vector.tensor_scalar_mul(
                            xcv[:], xcp_sb[dc][:, CPAD:], Wc(dc)[:, 3:4]
                        )
                        for k in (2, 1, 0):
                            s = K - 1 - k
                            nc.vector.scalar_tensor_tensor(
                                out=xcv[:, s:],
                                in0=xcp_sb[dc][:, CPAD : CPAD + L - s],
                                scalar=Wc(dc)[:, k : k + 1],
                                in1=xcv[:, s:],
                                op0=OP.mult,
                                op1=OP.add,
                            )
                        nc.scalar.activation(
                            wide(xs_sb, dc), xcv[:], AF.Silu,
                            bias=bconv(dc)[:, 0:1],
                        )

        # ---- GEMM B: dbc_T = W_xproj^T @ xs_T  (64 rows: dt_in | B | C) ----
        # dt_in rows stay fp32 for GEMM C; B/C rows are cast to bf16 and
        # round-tripped through DRAM for the partition-broadcast DMAs.
        with tc.tile_pool(name="psB", bufs=2, space="PSUM") as psB:
            for tb in range(NTB):
                ps = psB.tile([R + 2 * N, TB], f32, tag="psB", name="psB")
                for dc in range(NCH):
                    _mmb(
                        nc, ps[:], Wx(dc),
                        wide(xs_sb, dc, tb * TB, (tb + 1) * TB),
                        start=(dc == 0), stop=(dc == NCH - 1),
                    )
                sl = slice(tb * TB, (tb + 1) * TB)
                nc.scalar.activation(dtin_sb[:, sl], ps[0:R, :], AF.Copy)
                nc.scalar.activation(dbcb_sb[:, sl], ps[R : R + 2 * N, :], AF.Copy)
        nc.sync.dma_start(io["dbc_bf"][:, :], dbcb_sb[:])

        # ---- GEMM C: dt_T = softplus(W_dt^T @ dt_in_T + b_dt) ----
        # softplus(x) = ln(1 + exp(x)); exp and ln share one ACT table set.
        with tc.tile_pool(name="psC", bufs=2, space="PSUM") as psC, tc.tile_pool(
            name="spl", bufs=2
        ) as spl:
            for dc in range(NCH):
                for tb in range(NTB):
                    ps = psC.tile([P, TB], f32, tag="psC", name="psC")
                    _mmb(
                        nc, ps[:], Wdt_sb[:, dc * P : (dc + 1) * P],
                        dtin_sb[:, tb * TB : (tb + 1) * TB],
                        start=True, stop=True,
                    )
                    et = spl.tile([P, TB], f32, tag="et", name="et")
                    nc.scalar.activation(et[:], ps[:], AF.Exp, bias=bdt(dc)[:, 0:1])
                    nc.scalar.activation(
                        dt_sb[dc // 2][:, (dc % 2) * L + tb * TB : (dc % 2) * L + (tb + 1) * TB],
                        et[:], AF.Ln, bias=1.0,
                    )

        # u = dt * xs (bf16 out for the 2x dBx multiply)
        for h in range(2):
            nc.vector.tensor_mul(u_sb[h][:], dt_sb[h][:], xs_sb[h][:])

        # ---- selective scan: n outer, chunk-pair h inner ----
        with tc.tile_pool(name="scan", bufs=4) as sp, tc.tile_pool(
            name="rowp", bufs=3
        ) as rowp, tc.tile_pool(name="cbp", bufs=3) as cbp, tc.tile_pool(
            name="hcp", bufs=4
        ) as hcp, tc.tile_pool(name="psy", bufs=1, space="PSUM") as psy:

            y_ps = [psy.tile([P, W], f32, tag=f"y{h}", name=f"y{h}")
                    for h in range(2)]

            def build_bb(n):
                Bb = rowp.tile([P, L], BF16, tag="Bb", name="Bb")
                nc.sync.dma_start(
                    Bb[:], io["dbc_bf"][n : n + 1, :].partition_broadcast(P)
                )
                return Bb

            def build_cb(n):
                Cb = cbp.tile([P, L], BF16, tag="Cb", name="Cb")
                nc.sync.dma_start(
                    Cb[:],
                    io["dbc_bf"][N + n : N + n + 1, :].partition_broadcast(P),
                )
                return Cb

            for n in range(N):
                Bb, Cb = build_bb(n), build_cb(n)
                for h in range(2):
                    chunks = (2 * h, 2 * h + 1)
                    # dA over the pair; col L is persistently 0 so the scan
                    # restarts for the 2nd chunk
                    dA = dA_sb[h][n % 2]
                    nc.scalar.activation(
                        dA[:, 0:L], wide(dt_sb, chunks[0]), AF.Exp,
                        scale=A_sc(chunks[0])[:, n : n + 1],
                    )
                    nc.scalar.activation(
                        dA[:, L + 1 : W], wide(dt_sb, chunks[1], 1, L), AF.Exp,
                        scale=A_sc(chunks[1])[:, n : n + 1],
                    )

                    dBx = sp.tile([P, W], BF16, tag="dBx", name="dBx")
                    nc.vector.tensor_tensor(
                        dBx[:].rearrange("p (r f) -> p r f", r=2),
                        u_sb[h][:].rearrange("p (r f) -> p r f", r=2),
                        Bb[:].unsqueeze(1).broadcast_to((P, 2, L)),
                        op=OP.mult,
                    )
                    # scan in place: h overwrites dBx (fp32 internal state)
                    nc.vector.tensor_tensor_scan(
                        dBx[:], dA[:], dBx[:], 0.0, op0=OP.mult, op1=OP.add
                    )
                    hC = hcp.tile([P, W], BF16, tag="hC", name="hC")
                    hc_eng = nc.vector if (h, n) in HC_ON_DVE else nc.gpsimd
                    hc_eng.tensor_tensor(
                        hC[:].rearrange("p (r f) -> p r f", r=2),
                        dBx[:].rearrange("p (r f) -> p r f", r=2),
                        Cb[:].unsqueeze(1).broadcast_to((P, 2, L)),
                        op=OP.mult,
                    )
                    # y += hC via bf16 identity matmul (PSUM accumulate)
                    for tb in range(W // TB):
                        tsl = slice(tb * TB, (tb + 1) * TB)
                        _mmb(nc, y_ps[h][:, tsl], ident_sb[:], hC[:, tsl],
                             start=(n == 0), stop=(n == N - 1))

            # yz = (y + D*xs) * silu(z)
            for h in range(2):
                for dc in (2 * h, 2 * h + 1):
                    q = (dc % 2) * L
                    nc.vector.scalar_tensor_tensor(
                        out=wide(yz_sb, dc),
                        in0=wide(xs_sb, dc),
                        scalar=Dv(dc)[:, 0:1],
                        in1=y_ps[h][:, q : q + L],
                        op0=OP.mult,
                        op1=OP.add,
                    )
                    nc.vector.tensor_mul(
                        wide(yz_sb, dc), wide(yz_sb, dc), wide(zs_sb, dc)
                    )

        # ---- GEMM D: out_T = W_out^T @ yz_T ----
        with tc.tile_pool(name="psD", bufs=4, space="PSUM") as psD, tc.tile_pool(
            name="osb", bufs=2
        ) as osb:
            for mb in range(DM // P):
                ot = osb.tile([P, L], f32, tag="ot", name="ot")
                for tb in range(NTB):
                    ps = psD.tile([P, TB], f32, tag="psD", name="psD")
                    for dc in range(NCH):
                        _mmb(
                            nc, ps[:],
                            Wout_sb[:, dc, mb * P : (mb + 1) * P],
                            wide(yz_sb, dc, tb * TB, (tb + 1) * TB),
                            start=(dc == 0), stop=(dc == NCH - 1),
                        )
                    nc.scalar.activation(
                        ot[:, tb * TB : (tb + 1) * TB], ps[:], AF.Copy
                    )
                nc.sync.dma_start(
                    io["outT"][mb * P : (mb + 1) * P, :], ot[:]
                )


def build(reps=1):
    nc = bacc.Bacc(
        "TRN2",
        target_bir_lowering=False,
        debug=False,
        enable_asserts=False,
        num_devices=N_CORES,
    )
    SP_W = K + 1 + (R + 2 * N) + 1 + N + 1
    io = {
        "xT": nc.dram_tensor("xT", (P, NCH, L), F32R, kind="ExternalInput").ap(),
        "W_in": nc.dram_tensor("W_in", (P, NCH, 2 * DI), F32R, kind="ExternalInput").ap(),
        "small": nc.dram_tensor("small", (P, NCH, SP_W), F32, kind="ExternalInput").ap(),
        "Wdt": nc.dram_tensor("Wdt", (R, DI), BF16, kind="ExternalInput").ap(),
        "W_out": nc.dram_tensor("W_out", (P, NCH, DM), BF16, kind="ExternalInput").ap(),
        "Wdiag": nc.dram_tensor("Wdiag", (P, NCH * K * P), F32, kind="ExternalInput").ap(),
        "ident": nc.dram_tensor("ident", (P, P), BF16, kind="ExternalInput").ap(),
        "outT": nc.dram_tensor("outT", (DM, L), F32, kind="ExternalOutput").ap(),
        "dbc_bf": nc.dram_tensor("dbc_bf", (2 * N, L), BF16).ap(),
    }
    with tile.TileContext(nc) as tc:
        if reps == 1:
            emit_mamba(tc, io)
        else:
            with tc.For_i(0, reps, 1):
                emit_mamba(tc, io)
    nc.compile()
    return nc


_NC_CACHE = {}


def _get_nc(reps=1):
    if reps not in _NC_CACHE:
        _NC_CACHE[reps] = build(reps)
    return _NC_CACHE[reps]


def _chunked(a):
    """(DI, X) -> (P, NCH, X) with chunk dc = rows dc*P:(dc+1)*P."""
    return np.ascontiguousarray(
        np.asarray(a, np.float32).reshape(NCH, P, -1).transpose(1, 0, 2)
    )


def make_in_maps(inputs):
    import ml_dtypes

    x = np.asarray(inputs["x"], np.float32)
    SP_W = K + 1 + (R + 2 * N) + 1 + N + 1
    ident_bf = np.eye(P, dtype=ml_dtypes.bfloat16)
    in_maps = []
    for c in range(N_CORES):
        b = c % 4
        sfx = "f" if c < 4 else "b"
        xb = x[b] if c < 4 else x[b][::-1]

        def g(name):
            return np.asarray(inputs[f"{name}_{sfx}"], np.float32)

        Wc_ = g("W_conv")                      # (DI, K)
        small = np.concatenate(
            [
                Wc_,
                g("b_conv").reshape(DI, 1),
                g("W_xproj"),
                g("b_dt").reshape(DI, 1),
                -np.exp(g("A_log")),
                g("D").reshape(DI, 1),
            ],
            axis=1,
        )
        assert small.shape == (DI, SP_W)
        wdiag = np.zeros((P, NCH * K * P), np.float32)
        for dc in range(NCH):
            for k in range(K):
                blk = slice((dc * K + k) * P, (dc * K + k + 1) * P)
                np.fill_diagonal(wdiag[:, blk], Wc_[dc * P : (dc + 1) * P, k])

        in_maps.append(
            {
                "xT": _chunked(xb.T),
                "W_in": _chunked(g("W_in")),
                "small": _chunked(small),
                "Wdt": np.ascontiguousarray(g("W_dt")).astype(ml_dtypes.bfloat16),
                "W_out": _chunked(g("W_out")).astype(ml_dtypes.bfloat16),
                "Wdiag": wdiag,
                "ident": ident_bf,
            }
        )
    return in_maps


def assemble_output(results):
    out = np.empty((4, L, DM), np.float32)
    for b in range(4):
        of = results[b]["outT"].T
        ob = results[4 + b]["outT"].T[::-1]
        out[b] = of + ob
    return out


def kernel(**inputs):
    nc = _get_nc()
    in_maps = make_in_maps(inputs)
    res = run_bass_kernel_spmd(nc, in_maps, core_ids=list(range(N_CORES)))
    return assemble_output(res.results)
